# revision 7
# baseline (speedup 1.0000x reference)
"""AdaptiveLocal2DLayer forward on 8 TRN2 NeuronCores.

out[b, n] = sum_{c,h,w} x[b,c,h,w] * mask[h,w,n] * weights[c,h,w,n]
mask[h,w,n] = gy[h,n] * gx[w,n] * s[n]          (separable Gaussian)
s[n] = sqrt(H*W) / sqrt(sum_h gy^2 * sum_w gx^2)

Sharding: neuron axis N=1024 split over 8 cores (128 each). Weights are the
dominant traffic (bf16: 100MB chip-wide) and are read exactly once; x (3.1MB
bf16) is replicated. No collectives.

Per-core algorithm (everything on device except layout/cast prep):
  - x loaded once up front as [w, c, h, b] (one 3.1MB DMA)
  - gauss setup: gyT[n',h], per-neuron norm s[n'] from mu/sigma via DVE+ACT;
    gxu[w, g] for the core's 4 neuron subgroups (each subgroup of 32 neurons
    shares mu_x, so gx has only 4 distinct columns)
  - stream weights one h-block at a time ([w, c, hbs, n'] 1.5MB DMAs);
    mask-multiply by gx via 4 per-subgroup tensor_scalar ops (4x DVE mode);
    per (c,h) matmul(lhsT=gx*W [w,n'], rhs=x [w,b]) accumulated over c in
    PSUM; ACT copies PSUM->SBUF t1[n',h,b]; DVE applies gy*s and reduces
    over h per block.
"""

import numpy as np

import concourse.bass as bass
import concourse.mybir as mybir
import concourse.tile as tile
from concourse import bacc
from concourse.bass_utils import run_bass_kernel_spmd


B, C, H, W = 32, 3, 128, 128
N = 1024
NCORES = 8
NS = N // NCORES   # 128 neurons per core
NG = 4             # neuron subgroups per core (32 neurons each share mu_x)
GS = NS // NG

F32 = mybir.dt.float32
BF16 = mybir.dt.bfloat16
AF = mybir.ActivationFunctionType

# h-blocks: one weights DMA per block (C*hbs*NS bf16 = 1.5MB at hbs=16)
BLOCKS = [8] + [16] * 7 + [8]
assert sum(BLOCKS) == H

LAST_RESULT = None  # BassKernelResults stash for test harness
LAST_NC = None
LAST_IN_MAPS = None


def build_nc() -> bass.Bass:
    nc = bacc.Bacc("TRN2", target_bir_lowering=False)

    # chunk-linear flat layouts: one h-block chunk = one contiguous span
    wt_d = nc.dram_tensor("wt", [C * H * W * NS], BF16, kind="ExternalInput")
    xt_d = nc.dram_tensor("xt", [W * C * H * B], BF16, kind="ExternalInput")
    mux_d = nc.dram_tensor("mu_x", [NS], F32, kind="ExternalInput")
    muy_d = nc.dram_tensor("mu_y", [NS], F32, kind="ExternalInput")
    sgx_d = nc.dram_tensor("sigma_x", [NS], F32, kind="ExternalInput")
    sgy_d = nc.dram_tensor("sigma_y", [NS], F32, kind="ExternalInput")
    mux4_d = nc.dram_tensor("mux4", [NG], F32, kind="ExternalInput")
    sgx4_d = nc.dram_tensor("sgx4", [NG], F32, kind="ExternalInput")
    grid_d = nc.dram_tensor("grid", [W], F32, kind="ExternalInput")
    out_d = nc.dram_tensor("out", [NS, B], F32, kind="ExternalOutput")

    with tile.TileContext(nc) as tc:
        with (
            tc.tile_pool(name="singles", bufs=1) as singles,
            tc.tile_pool(name="wpool", bufs=3) as wpool,
            tc.tile_pool(name="ppool", bufs=7, space="PSUM") as ppool,
        ):
            # ---------------- x: one load, [w, c, h, b] ----------------
            xb = singles.tile([128, C * H * B], BF16)
            nc.scalar.dma_start(
                out=xb,
                in_=bass.AP(
                    tensor=xt_d, offset=0, ap=[[C * H * B, 128], [1, C * H * B]]
                ),
            )
            xbv = xb.rearrange("w (c h b) -> w c h b", c=C, h=H)

            # ---------------- mask construction ----------------
            grid_b = singles.tile([128, W], F32)
            nc.sync.dma_start(
                out=grid_b,
                in_=bass.AP(tensor=grid_d, offset=0, ap=[[0, 128], [1, W]]),
            )

            def col_load(dram):
                t = singles.tile([128, 1], F32, tag=f"col_{dram.name}")
                nc.sync.dma_start(
                    out=t, in_=bass.AP(tensor=dram, offset=0, ap=[[1, 128], [1, 1]])
                )
                return t

            mux_c = col_load(mux_d)
            muy_c = col_load(muy_d)
            sgx_c = col_load(sgx_d)
            sgy_c = col_load(sgy_d)

            inv_sgx = singles.tile([128, 1], F32)
            nc.vector.reciprocal(out=inv_sgx, in_=sgx_c)
            inv_sgy = singles.tile([128, 1], F32)
            nc.vector.reciprocal(out=inv_sgy, in_=sgy_c)

            # gT[n', u] = exp(-.5*((grid[u]-mu[n'])/sigma[n'])^2), row sum-sq
            def gauss(mu_c, inv_sg, tag):
                z = singles.tile([128, W], F32, tag=f"z_{tag}")
                nc.vector.tensor_scalar(
                    out=z, in0=grid_b, scalar1=mu_c, scalar2=inv_sg,
                    op0=mybir.AluOpType.subtract, op1=mybir.AluOpType.mult,
                )
                nc.vector.tensor_mul(out=z, in0=z, in1=z)
                g = singles.tile([128, W], F32, tag=f"g_{tag}")
                nc.scalar.activation(out=g, in_=z, func=AF.Exp, scale=-0.5)
                ssq = singles.tile([128, 1], F32, tag=f"ssq_{tag}")
                trash = singles.tile([128, W], F32, tag="trash")
                nc.scalar.activation(
                    out=trash, in_=g, func=AF.Square, accum_out=ssq
                )
                return g, ssq

            _, sx = gauss(mux_c, inv_sgx, "x")
            gyT, sy = gauss(muy_c, inv_sgy, "y")

            # s[n'] = sqrt(H*W)/sqrt(sx*sy) = 1/sqrt(sx*sy/(H*W))
            s_col = singles.tile([128, 1], F32)
            nc.vector.tensor_mul(out=s_col, in0=sx, in1=sy)
            nc.scalar.activation(
                out=s_col, in_=s_col, func=AF.Sqrt, scale=1.0 / (H * W)
            )
            nc.vector.reciprocal(out=s_col, in_=s_col)

            gysT = singles.tile([128, H], F32)  # [n', h] = gy^T * s
            nc.vector.tensor_scalar_mul(out=gysT, in0=gyT, scalar1=s_col)

            # gxu[w, g]: one gaussian column per neuron subgroup
            grid_col = col_load(grid_d)
            mu4_b = singles.tile([128, NG], F32)
            nc.sync.dma_start(
                out=mu4_b,
                in_=bass.AP(tensor=mux4_d, offset=0, ap=[[0, 128], [1, NG]]),
            )
            sg4_b = singles.tile([128, NG], F32)
            nc.sync.dma_start(
                out=sg4_b,
                in_=bass.AP(tensor=sgx4_d, offset=0, ap=[[0, 128], [1, NG]]),
            )
            inv4 = singles.tile([128, NG], F32)
            nc.vector.reciprocal(out=inv4, in_=sg4_b)
            zu = singles.tile([128, NG], F32)
            nc.vector.tensor_scalar(
                out=zu, in0=mu4_b, scalar1=grid_col, scalar2=None,
                op0=mybir.AluOpType.subtract,
            )
            nc.vector.tensor_mul(out=zu, in0=zu, in1=inv4)
            nc.vector.tensor_mul(out=zu, in0=zu, in1=zu)
            gxu = singles.tile([128, NG], F32)
            nc.scalar.activation(out=gxu, in_=zu, func=AF.Exp, scale=-0.5)

            # t1[n', h, b]: per-h matmul results; scaled by gy*s per block
            t1_sb = singles.tile([128, H, B], F32)
            out_acc = singles.tile([128, B], F32)

            # ---------------- main streaming loop ----------------
            h0 = 0
            for bi, hbs in enumerate(BLOCKS):
                wg = wpool.tile([128, C * hbs * NS], BF16, tag="wt")
                woff = h0 * C * W * NS
                nc.sync.dma_start(
                    out=wg,
                    in_=bass.AP(
                        tensor=wt_d, offset=woff,
                        ap=[[C * hbs * NS, 128], [1, C * hbs * NS]],
                    ),
                )
                wgv = wg.rearrange("w (c h n) -> w c h n", c=C, h=hbs)
                # mask multiply: per-subgroup per-partition scalar, 4x DVE mode
                for g in range(NG):
                    sl = wgv[:, :, :, g * GS : (g + 1) * GS]
                    nc.vector.tensor_scalar_mul(
                        out=sl, in0=sl, scalar1=gxu[:, g : g + 1]
                    )
                hg0 = 0
                while hg0 < hbs:
                    hg = min(8, hbs - hg0)
                    pt = ppool.tile([128, 8, B], F32)
                    for hl in range(hg):
                        for c in range(C):
                            nc.tensor.matmul(
                                pt[:, hl, :],
                                lhsT=wgv[:, c, hg0 + hl, :],
                                rhs=xbv[:, c, h0 + hg0 + hl, :],
                                start=(c == 0),
                                stop=(c == C - 1),
                            )
                    nc.scalar.activation(
                        out=t1_sb[:, h0 + hg0 : h0 + hg0 + hg, :],
                        in_=pt[:, :hg, :], func=AF.Copy,
                    )
                    hg0 += hg
                # scale this block by gy*s in place (overlaps the stream),
                # reduce over the block's h rows, accumulate
                gys_bc = gysT[:, h0 : h0 + hbs].rearrange(
                    "n (h o) -> n h o", o=1
                ).to_broadcast([128, hbs, B])
                t1h = t1_sb[:, h0 : h0 + hbs, :]
                nc.vector.tensor_mul(out=t1h, in0=t1h, in1=gys_bc)
                red_out = out_acc if bi == 0 else singles.tile(
                    [128, B], F32, tag=f"red_{bi}"
                )
                nc.vector.tensor_reduce(
                    out=red_out,
                    in_=t1h.rearrange("n h b -> n b h"),
                    axis=mybir.AxisListType.X,
                    op=mybir.AluOpType.add,
                )
                if bi > 0:
                    nc.vector.tensor_add(
                        out=out_acc, in0=out_acc, in1=red_out
                    )
                h0 += hbs

            nc.sync.dma_start(out=out_d[:, :], in_=out_acc)

    nc.compile()
    return nc


def prep_in_maps(x, mu_x, mu_y, sigma_x, sigma_y, weights):
    import ml_dtypes

    starts = []
    h0 = 0
    for hbs in BLOCKS:
        starts.append((h0, hbs))
        h0 += hbs

    # flat x: [w, c, h, b]
    xt = np.ascontiguousarray(
        np.transpose(x, (3, 1, 2, 0))
    ).astype(ml_dtypes.bfloat16).ravel()
    grid = np.linspace(0.0, 1.0, W, dtype=np.float32)
    in_maps = []
    for k in range(NCORES):
        sl = slice(k * NS, (k + 1) * NS)
        wsh = weights[:, :, :, sl].astype(ml_dtypes.bfloat16)  # [C,H,W,NS]
        # flat chunk-linear weights: chunk (h-block) is [w, c, hbs, NS]
        wt = np.concatenate(
            [
                np.transpose(wsh[:, h0 : h0 + hbs], (2, 0, 1, 3)).ravel()
                for h0, hbs in starts
            ]
        )
        in_maps.append(
            {
                "wt": wt,
                "xt": xt,
                "mu_x": np.ascontiguousarray(mu_x[sl]),
                "mu_y": np.ascontiguousarray(mu_y[sl]),
                "sigma_x": np.ascontiguousarray(sigma_x[sl]),
                "sigma_y": np.ascontiguousarray(sigma_y[sl]),
                "mux4": np.ascontiguousarray(mu_x[sl][::GS]),
                "sgx4": np.ascontiguousarray(sigma_x[sl][::GS]),
                "grid": grid,
            }
        )
    return in_maps


def kernel(x, mu_x, mu_y, sigma_x, sigma_y, weights):
    global LAST_RESULT
    x = np.asarray(x, dtype=np.float32)
    mu_x = np.asarray(mu_x, dtype=np.float32)
    mu_y = np.asarray(mu_y, dtype=np.float32)
    sigma_x = np.asarray(sigma_x, dtype=np.float32)
    sigma_y = np.asarray(sigma_y, dtype=np.float32)
    weights = np.asarray(weights, dtype=np.float32)

    global LAST_NC, LAST_IN_MAPS
    nc = build_nc()
    in_maps = prep_in_maps(x, mu_x, mu_y, sigma_x, sigma_y, weights)
    res = run_bass_kernel_spmd(nc, in_maps, core_ids=list(range(NCORES)))
    LAST_RESULT = res
    LAST_NC = nc
    LAST_IN_MAPS = in_maps
    full = np.concatenate([r["out"] for r in res.results], axis=0)  # [N, B]
    return np.ascontiguousarray(full.T).reshape(B, 1, 32, 32).astype(np.float32)


# revision 11
# speedup vs baseline: 1.1980x; 1.1980x over previous
"""AdaptiveLocal2DLayer forward on 8 TRN2 NeuronCores.

out[b, n] = sum_{c,h,w} x[b,c,h,w] * mask[h,w,n] * weights[c,h,w,n]
mask[h,w,n] = gy[h,n] * gx[w,n] * s[n]          (separable Gaussian)
s[n] = sqrt(H*W) / sqrt(sum_h gy^2 * sum_w gx^2)

Sharding: neuron axis N=1024 split over 8 cores (128 each). Weights are the
dominant traffic (bf16: 100MB chip-wide) and are read exactly once; x (3.1MB
bf16) is replicated. No collectives.

Per-core algorithm (everything on device except layout/cast prep):
  - x loaded once up front as [w, c, h, b] (one 3.1MB DMA)
  - gauss setup: gyT[n',h], per-neuron norm s[n'] from mu/sigma via DVE+ACT;
    gxu[w, g] for the core's 4 neuron subgroups (each subgroup of 32 neurons
    shares mu_x, so gx has only 4 distinct columns)
  - stream weights one h-block at a time ([w, c, hbs, n'] 1.5MB DMAs);
    mask-multiply by gx via 4 per-subgroup tensor_scalar ops (4x DVE mode);
    per (c,h) matmul(lhsT=gx*W [w,n'], rhs=x [w,b]) accumulated over c in
    PSUM; ACT copies PSUM->SBUF t1[n',h,b]; DVE applies gy*s and reduces
    over h per block.
"""

import numpy as np

import concourse.bass as bass
import concourse.mybir as mybir
import concourse.tile as tile
from concourse import bacc
from concourse.bass_utils import run_bass_kernel_spmd


B, C, H, W = 32, 3, 128, 128
N = 1024
NCORES = 8
NS = N // NCORES   # 128 neurons per core
NG = 4             # neuron subgroups per core (32 neurons each share mu_x)
GS = NS // NG

F32 = mybir.dt.float32
BF16 = mybir.dt.bfloat16
AF = mybir.ActivationFunctionType

# h-blocks: one weights DMA per block (C*hbs*NS bf16 = 1.5MB at hbs=16)
BLOCKS = [8] + [16] * 7 + [8]
assert sum(BLOCKS) == H

LAST_RESULT = None  # BassKernelResults stash for test harness
LAST_NC = None
LAST_IN_MAPS = None


def build_nc() -> bass.Bass:
    nc = bacc.Bacc("TRN2", target_bir_lowering=False)

    # chunk-linear flat layouts: one h-block chunk = one contiguous span
    wt_d = nc.dram_tensor("wt", [C * H * W * NS], BF16, kind="ExternalInput")
    xt_d = nc.dram_tensor("xt", [W * C * H * B], BF16, kind="ExternalInput")
    # all scalar setup constants packed into one [128, SK] host tensor:
    # per partition p: grid[0:W], mu_x[p], mu_y[p], sigma_x[p], sigma_y[p],
    # grid[p], mux4[0:4], sgx4[0:4]
    SK = W + 5 + 2 * NG
    setup_d = nc.dram_tensor("setup", [128 * SK], F32, kind="ExternalInput")
    out_d = nc.dram_tensor("out", [NS, B], F32, kind="ExternalOutput")

    with tile.TileContext(nc) as tc:
        with (
            tc.tile_pool(name="singles", bufs=1) as singles,
            tc.tile_pool(name="wpool", bufs=4) as wpool,
            tc.tile_pool(name="ppool", bufs=7, space="PSUM") as ppool,
        ):
            # first weight block: issue before everything else on sync so
            # the HBM stream starts during the (fixed) startup chain
            wg_first = wpool.tile([128, C * BLOCKS[0] * NS], BF16, tag="wt")
            nc.sync.dma_start(
                out=wg_first,
                in_=bass.AP(
                    tensor=wt_d, offset=0,
                    ap=[[C * BLOCKS[0] * NS, 128], [1, C * BLOCKS[0] * NS]],
                ),
            )

            # ---------------- x: one load, [w, c, h, b] ----------------
            xb = singles.tile([128, C * H * B], BF16)
            nc.scalar.dma_start(
                out=xb,
                in_=bass.AP(
                    tensor=xt_d, offset=0, ap=[[C * H * B, 128], [1, C * H * B]]
                ),
            )
            xbv = xb.rearrange("w (c h b) -> w c h b", c=C, h=H)

            # ---------------- mask construction ----------------
            setup_sb = singles.tile([128, SK], F32)
            nc.sync.dma_start(
                out=setup_sb,
                in_=bass.AP(tensor=setup_d, offset=0, ap=[[SK, 128], [1, SK]]),
            )
            grid_b = setup_sb[:, 0:W]
            mux_c = setup_sb[:, W : W + 1]
            muy_c = setup_sb[:, W + 1 : W + 2]
            sgx_c = setup_sb[:, W + 2 : W + 3]
            sgy_c = setup_sb[:, W + 3 : W + 4]
            grid_col = setup_sb[:, W + 4 : W + 5]
            mu4_b = setup_sb[:, W + 5 : W + 5 + NG]
            sg4_b = setup_sb[:, W + 5 + NG : W + 5 + 2 * NG]

            inv_sgx = singles.tile([128, 1], F32)
            nc.vector.reciprocal(out=inv_sgx, in_=sgx_c)
            inv_sgy = singles.tile([128, 1], F32)
            nc.vector.reciprocal(out=inv_sgy, in_=sgy_c)

            # gT[n', u] = exp(-.5*((grid[u]-mu[n'])/sigma[n'])^2), row sum-sq
            def gauss(mu_c, inv_sg, tag):
                z = singles.tile([128, W], F32, tag=f"z_{tag}")
                nc.vector.tensor_scalar(
                    out=z, in0=grid_b, scalar1=mu_c, scalar2=inv_sg,
                    op0=mybir.AluOpType.subtract, op1=mybir.AluOpType.mult,
                )
                nc.vector.tensor_mul(out=z, in0=z, in1=z)
                g = singles.tile([128, W], F32, tag=f"g_{tag}")
                nc.scalar.activation(out=g, in_=z, func=AF.Exp, scale=-0.5)
                ssq = singles.tile([128, 1], F32, tag=f"ssq_{tag}")
                trash = singles.tile([128, W], F32, tag="trash")
                nc.scalar.activation(
                    out=trash, in_=g, func=AF.Square, accum_out=ssq
                )
                return g, ssq

            _, sx = gauss(mux_c, inv_sgx, "x")
            gyT, sy = gauss(muy_c, inv_sgy, "y")

            # s[n'] = sqrt(H*W)/sqrt(sx*sy) = 1/sqrt(sx*sy/(H*W))
            s_col = singles.tile([128, 1], F32)
            nc.vector.tensor_mul(out=s_col, in0=sx, in1=sy)
            nc.scalar.activation(
                out=s_col, in_=s_col, func=AF.Sqrt, scale=1.0 / (H * W)
            )
            nc.vector.reciprocal(out=s_col, in_=s_col)

            gysT = singles.tile([128, H], F32)  # [n', h] = gy^T * s
            nc.vector.tensor_scalar_mul(out=gysT, in0=gyT, scalar1=s_col)

            # gxu[w, g]: one gaussian column per neuron subgroup
            inv4 = singles.tile([128, NG], F32)
            nc.vector.reciprocal(out=inv4, in_=sg4_b)
            zu = singles.tile([128, NG], F32)
            nc.vector.tensor_scalar(
                out=zu, in0=mu4_b, scalar1=grid_col, scalar2=None,
                op0=mybir.AluOpType.subtract,
            )
            nc.vector.tensor_mul(out=zu, in0=zu, in1=inv4)
            nc.vector.tensor_mul(out=zu, in0=zu, in1=zu)
            gxu = singles.tile([128, NG], F32)
            nc.scalar.activation(out=gxu, in_=zu, func=AF.Exp, scale=-0.5)

            # t1[n', h, b]: per-h matmul results; scaled by gy*s per block
            t1_sb = singles.tile([128, H, B], F32)
            out_acc = singles.tile([128, B], F32)

            # ---------------- main streaming loop ----------------
            h0 = 0
            for bi, hbs in enumerate(BLOCKS):
                if bi == 0:
                    wg = wg_first
                else:
                    wg = wpool.tile([128, C * hbs * NS], BF16, tag="wt")
                    woff = h0 * C * W * NS
                    nc.sync.dma_start(
                        out=wg,
                        in_=bass.AP(
                            tensor=wt_d, offset=woff,
                            ap=[[C * hbs * NS, 128], [1, C * hbs * NS]],
                        ),
                    )
                wgv = wg.rearrange("w (c h n) -> w c h n", c=C, h=hbs)
                # mask multiply: per-subgroup per-partition scalar.
                # 3 subgroups on DVE (tensor_scalar, 4x mode), 1 on ACT
                # (activation with per-partition scale) to balance engines.
                for g in range(NG):
                    sl = wgv[:, :, :, g * GS : (g + 1) * GS]
                    if g == NG - 1:
                        nc.scalar.activation(
                            out=sl, in_=sl, func=AF.Copy,
                            scale=gxu[:, g : g + 1],
                        )
                    else:
                        nc.vector.tensor_scalar_mul(
                            out=sl, in0=sl, scalar1=gxu[:, g : g + 1]
                        )
                hg0 = 0
                while hg0 < hbs:
                    hg = min(8, hbs - hg0)
                    pt = ppool.tile([128, 8, B], F32)
                    for hl in range(hg):
                        for c in range(C):
                            nc.tensor.matmul(
                                pt[:, hl, :],
                                lhsT=wgv[:, c, hg0 + hl, :],
                                rhs=xbv[:, c, h0 + hg0 + hl, :],
                                start=(c == 0),
                                stop=(c == C - 1),
                            )
                    nc.scalar.activation(
                        out=t1_sb[:, h0 + hg0 : h0 + hg0 + hg, :],
                        in_=pt[:, :hg, :], func=AF.Copy,
                    )
                    hg0 += hg
                # scale this block by gy*s in place (overlaps the stream),
                # reduce over the block's h rows, accumulate
                gys_bc = gysT[:, h0 : h0 + hbs].rearrange(
                    "n (h o) -> n h o", o=1
                ).to_broadcast([128, hbs, B])
                t1h = t1_sb[:, h0 : h0 + hbs, :]
                nc.vector.tensor_mul(out=t1h, in0=t1h, in1=gys_bc)
                red_out = out_acc if bi == 0 else singles.tile(
                    [128, B], F32, tag=f"red_{bi}"
                )
                nc.vector.tensor_reduce(
                    out=red_out,
                    in_=t1h.rearrange("n h b -> n b h"),
                    axis=mybir.AxisListType.X,
                    op=mybir.AluOpType.add,
                )
                if bi > 0:
                    nc.vector.tensor_add(
                        out=out_acc, in0=out_acc, in1=red_out
                    )
                h0 += hbs

            nc.sync.dma_start(out=out_d[:, :], in_=out_acc)

    nc.compile()
    return nc


def prep_in_maps(x, mu_x, mu_y, sigma_x, sigma_y, weights):
    import ml_dtypes

    starts = []
    h0 = 0
    for hbs in BLOCKS:
        starts.append((h0, hbs))
        h0 += hbs

    # flat x: [w, c, h, b]
    xt = np.ascontiguousarray(
        np.transpose(x, (3, 1, 2, 0))
    ).astype(ml_dtypes.bfloat16).ravel()
    grid = np.linspace(0.0, 1.0, W, dtype=np.float32)
    SK = W + 5 + 2 * NG
    in_maps = []
    for k in range(NCORES):
        sl = slice(k * NS, (k + 1) * NS)
        wsh = weights[:, :, :, sl].astype(ml_dtypes.bfloat16)  # [C,H,W,NS]
        # flat chunk-linear weights: chunk (h-block) is [w, c, hbs, NS]
        wt = np.concatenate(
            [
                np.transpose(wsh[:, h0 : h0 + hbs], (2, 0, 1, 3)).ravel()
                for h0, hbs in starts
            ]
        )
        setup = np.empty((128, SK), dtype=np.float32)
        setup[:, 0:W] = grid[None, :]
        setup[:, W] = mu_x[sl]
        setup[:, W + 1] = mu_y[sl]
        setup[:, W + 2] = sigma_x[sl]
        setup[:, W + 3] = sigma_y[sl]
        setup[:, W + 4] = grid
        setup[:, W + 5 : W + 5 + NG] = mu_x[sl][::GS][None, :]
        setup[:, W + 5 + NG : W + 5 + 2 * NG] = sigma_x[sl][::GS][None, :]
        in_maps.append({"wt": wt, "xt": xt, "setup": setup.ravel()})
    return in_maps


def kernel(x, mu_x, mu_y, sigma_x, sigma_y, weights):
    global LAST_RESULT
    x = np.asarray(x, dtype=np.float32)
    mu_x = np.asarray(mu_x, dtype=np.float32)
    mu_y = np.asarray(mu_y, dtype=np.float32)
    sigma_x = np.asarray(sigma_x, dtype=np.float32)
    sigma_y = np.asarray(sigma_y, dtype=np.float32)
    weights = np.asarray(weights, dtype=np.float32)

    global LAST_NC, LAST_IN_MAPS
    nc = build_nc()
    in_maps = prep_in_maps(x, mu_x, mu_y, sigma_x, sigma_y, weights)
    res = run_bass_kernel_spmd(nc, in_maps, core_ids=list(range(NCORES)))
    LAST_RESULT = res
    LAST_NC = nc
    LAST_IN_MAPS = in_maps
    full = np.concatenate([r["out"] for r in res.results], axis=0)  # [N, B]
    return np.ascontiguousarray(full.T).reshape(B, 1, 32, 32).astype(np.float32)
